# revision 5
# baseline (speedup 1.0000x reference)
"""Multi-head attention (B=4, T=2048, C=1024, 16 heads, no mask) on 8 TRN2 cores.

Sharding: pure query-sharding - core c handles batch b=c//2, query-half
ih=c%2 (1024 query rows). Each core computes K/V for its whole batch
(duplicated across the 2 cores sharing a batch) and its own Q rows, runs
full attention + output projection locally. Zero on-chip collectives.

Host-side prep (part of sharding): x[b] passed transposed (xT[c,t]) with the
core's query rows rotated to the front; weights passed transposed (W.T).
All matmul operands are float16 (fp32 PSUM accumulate): ~5e-4 rel err,
full-rate TensorE with pipelined weight loads and N=1024 moving operands.

Device dataflow per core:
  v16[t-tile][128, 16, 65] in SBUF: cols 0:64 = v head-slice, col 64 = 1.0
    (computed once via xT.T @ WvT, copied psum->sbuf; NO DRAM round-trip)
  kT[o,t]  = WkT_tile.T @ xT   qT[o,i] = WqT_tile.T @ xT[:, :1024]
  per head-pair p, j-tile g:
    scoresT[j,i1024] = kT_h.T-slice @ qT_h   (A/B row-packed tile_position)
    probsT = exp(scoresT/8)  (ScalarE, fp16 out)
    y[o,i] += v16[g][:,h,:].T @ probsT   (ones col -> row 64 = softmax denom)
  lazy denominator: DVE reciprocal of psum row 64 -> broadcast DMA ->
    y copied out of psum (frees banks for next pair) -> DVE multiply.
  out[i,:] = sum_p yT[p].T-slice @ WoT[p]
Projections for pair p+1 are emitted after attention(p) so the PE fills
exp-wait gaps with projection matmuls.
"""

import os
import ml_dtypes
import numpy as np

B, T, C = 4, 2048, 1024
NH, HS = 16, 64
N_CORES = 8

_CACHE = {}
LAST_RESULTS = {}


def _build_nc(debug_taps=False):
    import concourse.bass as bass
    import concourse.mybir as mybir
    import concourse.tile as tile
    from concourse import bacc

    F32 = mybir.dt.float32
    F16 = mybir.dt.bfloat16
    AF = mybir.ActivationFunctionType
    ALU = mybir.AluOpType

    nc = bacc.Bacc("TRN2", target_bir_lowering=False, debug=False, num_devices=N_CORES)

    xT = nc.dram_tensor("xT", [C, T], F16, kind="ExternalInput").ap()
    wkT = nc.dram_tensor("wkT", [C, C], F16, kind="ExternalInput").ap()
    wqT = nc.dram_tensor("wqT", [C, C], F16, kind="ExternalInput").ap()
    wvT = nc.dram_tensor("wvT", [C, C], F16, kind="ExternalInput").ap()
    woT = nc.dram_tensor("woT", [C, C], F16, kind="ExternalInput").ap()
    out = nc.dram_tensor("out", [1024, C], F32, kind="ExternalOutput").ap()

    with tile.TileContext(nc) as tc:
        with tc.tile_pool(name="sb", bufs=1) as sb, \
             tc.tile_pool(name="ps", bufs=1, space="PSUM") as ps:
            kT = [sb.tile([128, T], F16, tag=f"kT{p}", name=f"kT{p}") for p in range(8)]
            qT = [sb.tile([128, 1024], F16, tag=f"qT{p}", name=f"qT{p}") for p in range(8)]
            yTs = [sb.tile([128, 1024], F16, tag=f"yT{p}", name=f"yT{p}") for p in range(8)]
            xTs = [sb.tile([128, T], F16, tag=f"xT{c}", name=f"xT{c}") for c in range(8)]
            wos = [sb.tile([128, C], F16, tag=f"wo{p}", name=f"wo{p}") for p in range(8)]
            for c in range(8):
                nc.sync.dma_start(xTs[c][:], xT[c * 128:(c + 1) * 128, :])
            for p in range(8):
                nc.sync.dma_start(wos[p][:], woT[p * 128:(p + 1) * 128, :])

            # PSUM layout: sA(2 banks) + sB(2) + yA(2) + yB(2) = 8.
            # Projection and fin psum tiles share the sA/sB slots.

            # ---- v = x @ Wv.T  -> v16 tiles in SBUF [128, 16 heads, 65] ----
            # col 64 of each head slot = 1.0 (softmax-denominator ones column)
            v16 = [sb.tile([128, 16, 65], F16, tag=f"v16_{tt}", name=f"v16_{tt}")
                   for tt in range(16)]
            for tt in range(16):
                nc.vector.memset(v16[tt][:, :, 64:65], 1.0)
            wvh = [sb.tile([128, C], F16, tag=f"wv{c}", name=f"wv{c}") for c in range(8)]
            for c in range(8):
                nc.sync.dma_start(wvh[c][:], wvT[c * 128:(c + 1) * 128, :])
            for tt in range(16):
                vp = ps.tile([128, 16, 64], F32, tag=("sA" if tt % 2 == 0 else "sB"),
                             name=f"vps{tt}")
                for c in range(8):
                    for n2 in range(2):
                        nc.tensor.matmul(vp[:, n2 * 8:(n2 + 1) * 8, :],
                                         xTs[c][:, tt * 128:(tt + 1) * 128],
                                         wvh[c][:, n2 * 512:(n2 + 1) * 512],
                                         start=(c == 0), stop=(c == 7))
                nc.vector.tensor_copy(v16[tt][:, :, 0:64], vp[:])

            def proj_chunk(p, which, tag):
                # one PSUM accumulation group: kT half (which=0/1) or qT (=2)
                kp = ps.tile([128, 1024], F32, tag=tag, name=f"pj{p}_{which}")
                wsrc, wnm = (wkT, "wk") if which < 2 else (wqT, "wq")
                toff = 1024 if which == 1 else 0
                for c in range(8):
                    w = sb.tile([128, 128], F16, tag="wtile", bufs=6,
                                name=f"{wnm}{p}_{which}_{c}")
                    nc.sync.dma_start(w[:], wsrc[c * 128:(c + 1) * 128,
                                                 p * 128:(p + 1) * 128])
                    for n2 in range(2):
                        nc.tensor.matmul(
                            kp[:, n2 * 512:(n2 + 1) * 512], w[:],
                            xTs[c][:, toff + n2 * 512: toff + (n2 + 1) * 512],
                            start=(c == 0), stop=(c == 7))
                if which < 2:
                    nc.vector.tensor_copy(kT[p][:, toff:toff + 1024], kp[:])
                else:
                    nc.vector.tensor_copy(qT[p][:], kp[:])

            for which, tag in ((0, "sA"), (1, "sB"), (2, "sA")):
                proj_chunk(0, which, tag)

            for p in range(8):
                hA, hB = 2 * p, 2 * p + 1
                yA = ps.tile([65, 1024], F32, tag="yA", name=f"yA{p}")
                yB = ps.tile([65, 1024], F32, tag="yB", name=f"yB{p}")
                prAs, prBs = {}, {}
                for g in range(17):
                    # stage 1: scores + exp for j-tile g
                    if g < 16:
                        sA = ps.tile([128, 1024], F32, tag="sA", name=f"sA{p}_{g}")
                        sB = ps.tile([128, 1024], F32, tag="sB", name=f"sB{p}_{g}")
                        for n2 in range(2):
                            nc.tensor.matmul(sA[:, n2 * 512:(n2 + 1) * 512],
                                             kT[p][0:64, g * 128:(g + 1) * 128],
                                             qT[p][0:64, n2 * 512:(n2 + 1) * 512],
                                             start=True, stop=True)
                            nc.tensor.matmul(sB[:, n2 * 512:(n2 + 1) * 512],
                                             kT[p][64:128, g * 128:(g + 1) * 128],
                                             qT[p][64:128, n2 * 512:(n2 + 1) * 512],
                                             start=True, stop=True)
                        prA = sb.tile([128, 1024], F16, tag="prA", bufs=3,
                                      name=f"prA{p}_{g}")
                        prB = sb.tile([128, 1024], F16, tag="prB", bufs=3,
                                      name=f"prB{p}_{g}")
                        nc.scalar.activation(prA[:], sA[:], AF.Exp, scale=0.125)
                        nc.scalar.activation(prB[:], sB[:], AF.Exp, scale=0.125)
                        prAs[g], prBs[g] = prA, prB
                    # next pair's projections as dense PE bursts mid-attention
                    if p < 7 and g in (5, 10, 15):
                        proj_chunk(p + 1, (0, 1, 2)[(5, 10, 15).index(g)],
                                   ("sA", "sB", "sA")[(5, 10, 15).index(g)])
                    # stage 2: AV for j-tile g-1 (its exp finished last round)
                    if g > 0:
                        gg = g - 1
                        for n2 in range(2):
                            nc.tensor.matmul(yA[:, n2 * 512:(n2 + 1) * 512],
                                             v16[gg][:, hA, :],
                                             prAs[gg][:, n2 * 512:(n2 + 1) * 512],
                                             start=(gg == 0), stop=(gg == 15))
                            nc.tensor.matmul(yB[:, n2 * 512:(n2 + 1) * 512],
                                             v16[gg][:, hB, :],
                                             prBs[gg][:, n2 * 512:(n2 + 1) * 512],
                                             start=(gg == 0), stop=(gg == 15))

                # lazy softmax denominators: reciprocal straight off the psum
                # denominator row + copy y out of psum (frees banks for the
                # next pair) run first on DVE; the broadcast DMA and the
                # final multiply (GpSimd) trail off the critical path.
                rrecs, yraws = [], []
                for head, yy in ((0, yA), (1, yB)):
                    rrec = sb.tile([1, 1024], F16, tag="rrec", bufs=2,
                                   name=f"rrec{p}_{head}")
                    with nc.allow_low_precision(reason="softmax denom bf16"):
                        nc.vector.reciprocal(rrec[:], yy[64:65, :])
                    yraw = sb.tile([64, 1024], F16, tag="yraw", bufs=2,
                                   name=f"yraw{p}_{head}")
                    nc.vector.tensor_copy(yraw[:], yy[0:64, :])
                    rrecs.append(rrec)
                    yraws.append(yraw)
                for head in (0, 1):
                    bc = sb.tile([64, 1024], F16, tag="bc", bufs=2,
                                 name=f"bc{p}_{head}")
                    nc.sync.dma_start(
                        bc[:], rrecs[head][:].unsqueeze(1).to_broadcast((1, 64, 1024)))
                    nc.gpsimd.tensor_tensor(
                        out=yTs[p][head * 64:(head + 1) * 64, :],
                        in0=yraws[head][:], in1=bc[:], op=ALU.mult)

            # ---- final projection ----
            for it in range(8):
                fp_ = ps.tile([128, 1024], F32, tag=("sA" if it % 2 == 0 else "sB"),
                              name=f"fin{it}")
                for p in range(8):
                    for n2 in range(2):
                        nc.tensor.matmul(fp_[:, n2 * 512:(n2 + 1) * 512],
                                         yTs[p][:, it * 128:(it + 1) * 128],
                                         wos[p][:, n2 * 512:(n2 + 1) * 512],
                                         start=(p == 0), stop=(p == 7))
                ob = sb.tile([128, 1024], F32, tag="ob", bufs=2, name=f"ob{it}")
                nc.vector.tensor_copy(ob[:], fp_[:])
                nc.sync.dma_start(out[it * 128:(it + 1) * 128, :], ob[:])

    nc.compile()
    return nc


def _get_nc():
    if "nc" not in _CACHE:
        _CACHE["nc"] = _build_nc()
    return _CACHE["nc"]


def _make_in_maps(x, Wk, Wq, Wv, Wo):
    wkT = np.ascontiguousarray(Wk.T).astype(ml_dtypes.bfloat16)
    wqT = np.ascontiguousarray(Wq.T).astype(ml_dtypes.bfloat16)
    wvT = np.ascontiguousarray(Wv.T).astype(ml_dtypes.bfloat16)
    woT = np.ascontiguousarray(Wo.T).astype(ml_dtypes.bfloat16)
    in_maps = []
    for core in range(N_CORES):
        b, ih = core // 2, core % 2
        xb = np.asarray(x[b], dtype=np.float32)
        if ih == 0:
            xloc = xb
        else:
            xloc = np.concatenate([xb[1024:], xb[:1024]], axis=0)
        in_maps.append({
            "xT": np.ascontiguousarray(xloc.T).astype(ml_dtypes.bfloat16),
            "wkT": wkT, "wqT": wqT, "wvT": wvT, "woT": woT,
        })
    return in_maps


def _install_ntff_hook_shim():
    import sys, types
    try:
        from antenv.axon_hooks import get_axon_ntff_profile_hook  # noqa
        return True
    except ImportError:
        pass
    try:
        sys.path.insert(0, "/root/.axon_site")
        from trn_agent_boot.trn_boot import _ntff_profile_via_ctypes
        hook = _ntff_profile_via_ctypes("/opt/axon/libaxon_pjrt.so")
        if hook is None:
            return False
        mod = types.ModuleType("antenv.axon_hooks")
        mod._hook = hook
        mod.get_axon_ntff_profile_hook = lambda: mod._hook
        mod.set_axon_ntff_profile_hook = lambda h: setattr(mod, "_hook", h)
        sys.modules["antenv.axon_hooks"] = mod
        import antenv
        antenv.axon_hooks = mod
        return True
    except Exception:
        return False


def kernel(x, Wk, Wq, Wv, Wo):
    from concourse.bass_utils import run_bass_kernel_spmd

    nc = _get_nc()
    in_maps = _make_in_maps(x, Wk, Wq, Wv, Wo)
    trace = bool(int(os.environ.get("ATT_TRACE", "0")))
    if trace and not _install_ntff_hook_shim():
        trace = False
    res = run_bass_kernel_spmd(nc, in_maps, core_ids=list(range(N_CORES)),
                               trace=trace)
    LAST_RESULTS["exec_time_ns"] = res.exec_time_ns
    LAST_RESULTS["res"] = res
    full = np.empty((B, T, C), dtype=np.float32)
    for core in range(N_CORES):
        b, ih = core // 2, core % 2
        full[b, ih * 1024:(ih + 1) * 1024] = res.results[core]["out"]
    return full


# revision 7
# speedup vs baseline: 1.2255x; 1.2255x over previous
"""Multi-head attention (B=4, T=2048, C=1024, 16 heads, no mask) on 8 TRN2 cores.

Sharding: pure query-sharding - core c handles batch b=c//2, query-half
ih=c%2 (1024 query rows). Each core computes K/V for its whole batch
(duplicated across the 2 cores sharing a batch) and its own Q rows, runs
full attention + output projection locally. Zero on-chip collectives.

Host-side prep (part of sharding): x[b] passed transposed (xT[c,t]) with the
core's query rows rotated to the front; weights passed transposed (W.T).
All matmul operands are float16 (fp32 PSUM accumulate): ~5e-4 rel err,
full-rate TensorE with pipelined weight loads and N=1024 moving operands.

Device dataflow per core:
  v16[t-tile][128, 16, 65] in SBUF: cols 0:64 = v head-slice, col 64 = 1.0
    (computed once via xT.T @ WvT, copied psum->sbuf; NO DRAM round-trip)
  kT[o,t]  = WkT_tile.T @ xT   qT[o,i] = WqT_tile.T @ xT[:, :1024]
  per head-pair p, j-tile g:
    scoresT[j,i1024] = kT_h.T-slice @ qT_h   (A/B row-packed tile_position)
    probsT = exp(scoresT/8)  (ScalarE, fp16 out)
    y[o,i] += v16[g][:,h,:].T @ probsT   (ones col -> row 64 = softmax denom)
  lazy denominator: DVE reciprocal of psum row 64 -> broadcast DMA ->
    y copied out of psum (frees banks for next pair) -> DVE multiply.
  out[i,:] = sum_p yT[p].T-slice @ WoT[p]
Projections for pair p+1 are emitted after attention(p) so the PE fills
exp-wait gaps with projection matmuls.
"""

import os
import ml_dtypes
import numpy as np

B, T, C = 4, 2048, 1024
NH, HS = 16, 64
N_CORES = 8

_CACHE = {}
LAST_RESULTS = {}


def _build_nc(debug_taps=False):
    import concourse.bass as bass
    import concourse.mybir as mybir
    import concourse.tile as tile
    from concourse import bacc

    F32 = mybir.dt.float32
    F16 = mybir.dt.bfloat16
    AF = mybir.ActivationFunctionType
    ALU = mybir.AluOpType

    nc = bacc.Bacc("TRN2", target_bir_lowering=False, debug=False, num_devices=N_CORES)

    xT = nc.dram_tensor("xT", [C, T], F16, kind="ExternalInput").ap()
    wkT = nc.dram_tensor("wkT", [C, C], F16, kind="ExternalInput").ap()
    wqT = nc.dram_tensor("wqT", [C, C], F16, kind="ExternalInput").ap()
    wvT = nc.dram_tensor("wvT", [C, C], F16, kind="ExternalInput").ap()
    woT = nc.dram_tensor("woT", [C, C], F16, kind="ExternalInput").ap()
    out = nc.dram_tensor("out", [1024, C], F32, kind="ExternalOutput").ap()

    with tile.TileContext(nc) as tc:
        with tc.tile_pool(name="sb", bufs=1) as sb, \
             tc.tile_pool(name="ps", bufs=1, space="PSUM") as ps:
            kT = [sb.tile([128, T], F16, tag=f"kT{p}", name=f"kT{p}") for p in range(8)]
            qT = [sb.tile([128, 1024], F16, tag=f"qT{p}", name=f"qT{p}") for p in range(8)]
            yTs = [sb.tile([128, 1024], F16, tag=f"yT{p}", name=f"yT{p}") for p in range(8)]
            xTs = [sb.tile([128, T], F16, tag=f"xT{c}", name=f"xT{c}") for c in range(8)]
            wos = [sb.tile([128, C], F16, tag=f"wo{p}", name=f"wo{p}") for p in range(8)]
            for c in range(8):
                nc.sync.dma_start(xTs[c][:], xT[c * 128:(c + 1) * 128, :])
            for p in range(8):
                nc.sync.dma_start(wos[p][:], woT[p * 128:(p + 1) * 128, :])

            # PSUM layout: sA(2 banks) + sB(2) + yA(2) + yB(2) = 8.
            # Projection and fin psum tiles share the sA/sB slots.

            # ---- v = x @ Wv.T  -> v16 tiles in SBUF [128, 16 heads, 65] ----
            # col 64 of each head slot = 1.0 (softmax-denominator ones column)
            v16 = [sb.tile([128, 16, 65], F16, tag=f"v16_{tt}", name=f"v16_{tt}")
                   for tt in range(16)]
            for tt in range(16):
                nc.vector.memset(v16[tt][:, :, 64:65], 1.0)
            wvh = [sb.tile([128, C], F16, tag=f"wv{c}", name=f"wv{c}") for c in range(8)]
            for c in range(8):
                nc.sync.dma_start(wvh[c][:], wvT[c * 128:(c + 1) * 128, :])
            for tt in range(16):
                vp = ps.tile([128, 16, 64], F32, tag=("sA" if tt % 2 == 0 else "sB"),
                             name=f"vps{tt}")
                for c in range(8):
                    for n2 in range(2):
                        nc.tensor.matmul(vp[:, n2 * 8:(n2 + 1) * 8, :],
                                         xTs[c][:, tt * 128:(tt + 1) * 128],
                                         wvh[c][:, n2 * 512:(n2 + 1) * 512],
                                         start=(c == 0), stop=(c == 7))
                nc.vector.tensor_copy(v16[tt][:, :, 0:64], vp[:])

            def proj_chunk(p, which, tag):
                # one PSUM accumulation group: kT half (which=0/1) or qT (=2)
                kp = ps.tile([128, 1024], F32, tag=tag, name=f"pj{p}_{which}")
                wsrc, wnm = (wkT, "wk") if which < 2 else (wqT, "wq")
                toff = 1024 if which == 1 else 0
                for c in range(8):
                    w = sb.tile([128, 128], F16, tag="wtile", bufs=6,
                                name=f"{wnm}{p}_{which}_{c}")
                    nc.sync.dma_start(w[:], wsrc[c * 128:(c + 1) * 128,
                                                 p * 128:(p + 1) * 128])
                    for n2 in range(2):
                        nc.tensor.matmul(
                            kp[:, n2 * 512:(n2 + 1) * 512], w[:],
                            xTs[c][:, toff + n2 * 512: toff + (n2 + 1) * 512],
                            start=(c == 0), stop=(c == 7))
                if which < 2:
                    nc.vector.tensor_copy(kT[p][:, toff:toff + 1024], kp[:])
                else:
                    nc.vector.tensor_copy(qT[p][:], kp[:])

            for which, tag in ((0, "sA"), (1, "sB"), (2, "sA")):
                proj_chunk(0, which, tag)

            for p in range(8):
                hA, hB = 2 * p, 2 * p + 1
                yA = ps.tile([65, 1024], F32, tag="yA", name=f"yA{p}")
                yB = ps.tile([65, 1024], F32, tag="yB", name=f"yB{p}")
                prAs, prBs = {}, {}
                for g in range(17):
                    # stage 1: scores + exp for j-tile g
                    if g < 16:
                        sA = ps.tile([128, 1024], F32, tag="sA", name=f"sA{p}_{g}")
                        sB = ps.tile([128, 1024], F32, tag="sB", name=f"sB{p}_{g}")
                        for n2 in range(2):
                            nc.tensor.matmul(sA[:, n2 * 512:(n2 + 1) * 512],
                                             kT[p][0:64, g * 128:(g + 1) * 128],
                                             qT[p][0:64, n2 * 512:(n2 + 1) * 512],
                                             start=True, stop=True)
                            nc.tensor.matmul(sB[:, n2 * 512:(n2 + 1) * 512],
                                             kT[p][64:128, g * 128:(g + 1) * 128],
                                             qT[p][64:128, n2 * 512:(n2 + 1) * 512],
                                             start=True, stop=True)
                        prA = sb.tile([128, 1024], F16, tag="prA", bufs=3,
                                      name=f"prA{p}_{g}")
                        prB = sb.tile([128, 1024], F16, tag="prB", bufs=3,
                                      name=f"prB{p}_{g}")
                        nc.scalar.activation(prA[:], sA[:], AF.Exp, scale=0.125)
                        nc.scalar.activation(prB[:], sB[:], AF.Exp, scale=0.125)
                        prAs[g], prBs[g] = prA, prB
                    # next pair's projections as dense PE bursts mid-attention
                    if p < 7 and g in (5, 10, 15):
                        proj_chunk(p + 1, (0, 1, 2)[(5, 10, 15).index(g)],
                                   ("sA", "sB", "sA")[(5, 10, 15).index(g)])
                    # stage 2: AV for j-tile g-1 (its exp finished last round)
                    if g > 0:
                        gg = g - 1
                        for n2 in range(2):
                            nc.tensor.matmul(yA[:, n2 * 512:(n2 + 1) * 512],
                                             v16[gg][:, hA, :],
                                             prAs[gg][:, n2 * 512:(n2 + 1) * 512],
                                             start=(gg == 0), stop=(gg == 15))
                            nc.tensor.matmul(yB[:, n2 * 512:(n2 + 1) * 512],
                                             v16[gg][:, hB, :],
                                             prBs[gg][:, n2 * 512:(n2 + 1) * 512],
                                             start=(gg == 0), stop=(gg == 15))

                # lazy softmax denominators: y rows leave psum via DVE casts
                # and the denominator rows via ScalarE copies (psum banks are
                # free for the next pair within ~3us); the spray-reciprocal,
                # broadcast DMA and GpSimd multiply trail off the critical
                # path (nothing waits on them until the final projection).
                yraws, drows = [], []
                for head, yy in ((0, yA), (1, yB)):
                    yraw = sb.tile([64, 1024], F16, tag="yraw", bufs=2,
                                   name=f"yraw{p}_{head}")
                    nc.vector.tensor_copy(yraw[:], yy[0:64, :])
                    yraws.append(yraw)
                for head, yy in ((0, yA), (1, yB)):
                    drow = sb.tile([1, 1024], F32, tag="drow", bufs=2,
                                   name=f"drow{p}_{head}")
                    nc.scalar.copy(drow[:], yy[64:65, :])
                    drows.append(drow)
                for head in (0, 1):
                    rsp = sb.tile([128, 8], F32, tag="rsp", bufs=2,
                                  name=f"rsp{p}_{head}")
                    nc.sync.dma_start(rsp[:], drows[head][:])
                    rrec = sb.tile([128, 8], F16, tag="rrec", bufs=2,
                                   name=f"rrec{p}_{head}")
                    with nc.allow_low_precision(reason="softmax denom bf16"):
                        nc.vector.reciprocal(rrec[:], rsp[:])
                    rrow = sb.tile([1, 1024], F16, tag="rrow", bufs=2,
                                   name=f"rrow{p}_{head}")
                    nc.sync.dma_start(rrow[:], rrec[:])
                    bc = sb.tile([64, 1024], F16, tag="bc", bufs=2,
                                 name=f"bc{p}_{head}")
                    nc.sync.dma_start(
                        bc[:], rrow[:].unsqueeze(1).to_broadcast((1, 64, 1024)))
                    nc.gpsimd.tensor_tensor(
                        out=yTs[p][head * 64:(head + 1) * 64, :],
                        in0=yraws[head][:], in1=bc[:], op=ALU.mult)

            # ---- final projection ----
            for it in range(8):
                fp_ = ps.tile([128, 1024], F32, tag=("sA" if it % 2 == 0 else "sB"),
                              name=f"fin{it}")
                for p in range(8):
                    for n2 in range(2):
                        nc.tensor.matmul(fp_[:, n2 * 512:(n2 + 1) * 512],
                                         yTs[p][:, it * 128:(it + 1) * 128],
                                         wos[p][:, n2 * 512:(n2 + 1) * 512],
                                         start=(p == 0), stop=(p == 7))
                ob = sb.tile([128, 1024], F32, tag="ob", bufs=2, name=f"ob{it}")
                nc.vector.tensor_copy(ob[:], fp_[:])
                nc.sync.dma_start(out[it * 128:(it + 1) * 128, :], ob[:])

    nc.compile()
    return nc


def _get_nc():
    if "nc" not in _CACHE:
        _CACHE["nc"] = _build_nc()
    return _CACHE["nc"]


def _make_in_maps(x, Wk, Wq, Wv, Wo):
    wkT = np.ascontiguousarray(Wk.T).astype(ml_dtypes.bfloat16)
    wqT = np.ascontiguousarray(Wq.T).astype(ml_dtypes.bfloat16)
    wvT = np.ascontiguousarray(Wv.T).astype(ml_dtypes.bfloat16)
    woT = np.ascontiguousarray(Wo.T).astype(ml_dtypes.bfloat16)
    in_maps = []
    for core in range(N_CORES):
        b, ih = core // 2, core % 2
        xb = np.asarray(x[b], dtype=np.float32)
        if ih == 0:
            xloc = xb
        else:
            xloc = np.concatenate([xb[1024:], xb[:1024]], axis=0)
        in_maps.append({
            "xT": np.ascontiguousarray(xloc.T).astype(ml_dtypes.bfloat16),
            "wkT": wkT, "wqT": wqT, "wvT": wvT, "woT": woT,
        })
    return in_maps


def _install_ntff_hook_shim():
    import sys, types
    try:
        from antenv.axon_hooks import get_axon_ntff_profile_hook  # noqa
        return True
    except ImportError:
        pass
    try:
        sys.path.insert(0, "/root/.axon_site")
        from trn_agent_boot.trn_boot import _ntff_profile_via_ctypes
        hook = _ntff_profile_via_ctypes("/opt/axon/libaxon_pjrt.so")
        if hook is None:
            return False
        mod = types.ModuleType("antenv.axon_hooks")
        mod._hook = hook
        mod.get_axon_ntff_profile_hook = lambda: mod._hook
        mod.set_axon_ntff_profile_hook = lambda h: setattr(mod, "_hook", h)
        sys.modules["antenv.axon_hooks"] = mod
        import antenv
        antenv.axon_hooks = mod
        return True
    except Exception:
        return False


def kernel(x, Wk, Wq, Wv, Wo):
    from concourse.bass_utils import run_bass_kernel_spmd

    nc = _get_nc()
    in_maps = _make_in_maps(x, Wk, Wq, Wv, Wo)
    trace = bool(int(os.environ.get("ATT_TRACE", "0")))
    if trace and not _install_ntff_hook_shim():
        trace = False
    res = run_bass_kernel_spmd(nc, in_maps, core_ids=list(range(N_CORES)),
                               trace=trace)
    LAST_RESULTS["exec_time_ns"] = res.exec_time_ns
    LAST_RESULTS["res"] = res
    full = np.empty((B, T, C), dtype=np.float32)
    for core in range(N_CORES):
        b, ih = core // 2, core % 2
        full[b, ih * 1024:(ih + 1) * 1024] = res.results[core]["out"]
    return full
